# revision 63
# baseline (speedup 1.0000x reference)
"""MoE layer (top-2 of 8 experts) Trainium2 kernel, expert-parallel on 8 cores.

Strategy
--------
Host: computes the router (logits -> softmax -> top-2) in float64, builds the
per-expert token dispatch (capacity C = NTB*128 with zero-weight padding),
gathers and lays out per-core inputs (bf16) for DMA-friendly access, and
scatter-adds the per-expert partial outputs back into the full output.

Device (per core, expert e): y = (gelu(x @ w1 + b1) @ w2 + b2) * w_combine
for the C tokens routed to the core's expert, all matmuls in bf16 (fp32
PSUM accumulation). Both weight matrices live in SBUF for the whole kernel
(one DMA pass, 16 MB bf16 total), so HBM traffic is ~34 MB/core instead of
~120 MB. bf16 also gets FWL weight loads on the PE (2 elem/cycle) that the
reorder window hides behind the previous matmul, which fp32r cannot do.

Tiling: tokens in chunks of [384 x 5, 256] (all matmul free dims >= 256 so
FWL weight loads hide behind the previous matmul). GEMM1 produces
hid [f128, tc] tiles (F on partitions) via 8 accumulating matmuls each,
gelu+b1 fused on ScalarE with bf16 output, rotating 2 PSUM banks. GEMM2
contracts over F in a single f-sweep with nb*2 concurrent PSUM
accumulation groups (6 banks; 2+6 = all 8); each hid lhsT tile feeds its
two h-half matmuls back-to-back, so the second skips the
stationary-operand reload. This pairing puts the LDWEIGHTS+MATMUL stream
into its pipelined regime (~131 ns per N=512 matmul vs ~253 unpaired) and
measured ~1.5x end-to-end on HW. NOTE: the regime is fragile — a unified
8-bank PSUM pool (CBLK=4, GEMM1 rotating all 8 banks) re-measured at
506us vs 345us because the extra PSUM-slot waits drain the PE queue that
the LDWEIGHTS pull-ahead depends on. b2 added on VectorE, combine-weight
on ScalarE during PSUM evacuation. b2 is partition-broadcast on device
via a K=1 ones-vector matmul (a partition_broadcast DMA costs ~114us of
sequencer descriptor time).
"""

import numpy as np

# ---------------------------------------------------------------- constants
B, S, H, F, E, TOP_K = 4, 2048, 1024, 4096, 8, 2
T = B * S
NH = H // 128          # 8 h-blocks
NF = F // 128          # 32 f-tiles
NTB_DEFAULT = 17       # token blocks of 128 -> capacity 2176 (mean load 2048)
CBLK = 3               # token blocks per chunk (384 tokens)

_CACHE = {}


def _chunks(ntb):
    """List of (block_start, n_blocks) chunks covering ntb token blocks.
    Avoids a trailing 1-block chunk (N=128 matmuls cannot hide their
    weight loads): 17 -> [3, 3, 3, 3, 3, 2]."""
    out = []
    b = 0
    while b < ntb:
        rem = ntb - b
        if rem == CBLK + 1:
            nb = CBLK - 1
        else:
            nb = min(CBLK, rem)
        out.append((b, nb))
        b += nb
    return out


def _build_nc(ntb=NTB_DEFAULT, loop_r=None, dram_internal=False, stages="full"):
    """loop_r=None: real kernel. loop_r=R: perf variant, For_i repeats the
    whole body R times. dram_internal: perf-only - inputs live in Internal
    DRAM (initialized on device) so timing calls ship no data."""
    import concourse.mybir as mybir
    import concourse.bass as bass
    from concourse import bacc
    from concourse.tile import TileContext
    from contextlib import ExitStack

    F32 = mybir.dt.float32
    BF16 = mybir.dt.bfloat16
    AFT = mybir.ActivationFunctionType

    C = ntb * 128
    chunks = _chunks(ntb)
    XCOLS = sum(NH * nb * 128 for _, nb in chunks)

    nc = bacc.Bacc(None, target_bir_lowering=False)

    kind = "Internal" if dram_internal else "ExternalInput"
    # DRAM tensors (host-prepared layouts; see _prep_core_inputs)
    xr_d = nc.dram_tensor("xr", [128, XCOLS], BF16, kind=kind)
    w1r_d = nc.dram_tensor("w1r", [128, NF * NH * 128], BF16, kind=kind)
    w2r_d = nc.dram_tensor("w2r", [128, NF * H], BF16, kind=kind)
    b1_d = nc.dram_tensor("b1", [F], F32, kind=kind)
    b2_d = nc.dram_tensor("b2", [H], F32, kind=kind)
    wc_d = nc.dram_tensor("wc", [128, ntb], F32, kind=kind)
    if dram_internal:
        dummy_d = nc.dram_tensor("perfdummy", [1, 1], F32, kind="ExternalInput")
        y_d = nc.dram_tensor("y", [C, H], F32, kind="Internal")
        ydum_d = nc.dram_tensor("ydum", [128, 8], F32, kind="ExternalOutput")
    else:
        y_d = nc.dram_tensor("y", [C, H], F32, kind="ExternalOutput")

    with TileContext(nc) as tc:
        with ExitStack() as stk:
            cpool = stk.enter_context(tc.tile_pool(name="consts", bufs=1))
            w1p = stk.enter_context(tc.tile_pool(name="w1p", bufs=1))
            w2p = stk.enter_context(tc.tile_pool(name="w2p", bufs=1))
            hidp = stk.enter_context(tc.tile_pool(name="hidp", bufs=1))
            xp = stk.enter_context(tc.tile_pool(name="xp", bufs=2))
            outp = stk.enter_context(tc.tile_pool(name="outp", bufs=4))
            ps1p = stk.enter_context(tc.tile_pool(name="ps1", bufs=2, space="PSUM"))
            ps2p = stk.enter_context(tc.tile_pool(name="ps2", bufs=6, space="PSUM"))

            if dram_internal:
                # device-side init of Internal inputs (avoid NaN garbage)
                with tc.tile_pool(name="initp", bufs=1) as initp:
                    zb = initp.tile([128, 4096], BF16, name="zb")
                    nc.vector.memset(zb, 0.01)
                    zf = initp.tile([128, 1024], F32, name="zf")
                    nc.vector.memset(zf, 0.01)
                    for col in range(0, XCOLS, 4096):
                        w = min(4096, XCOLS - col)
                        nc.sync.dma_start(out=xr_d[:, col : col + w], in_=zb[:, :w])
                    for col in range(0, NF * NH * 128, 4096):
                        nc.scalar.dma_start(
                            out=w1r_d[:, col : col + 4096], in_=zb[:, :4096]
                        )
                    for col in range(0, NF * H, 4096):
                        nc.gpsimd.dma_start(
                            out=w2r_d[:, col : col + 4096], in_=zb[:, :4096]
                        )
                    nc.sync.dma_start(
                        out=b1_d.rearrange("(t p) -> p t", p=128), in_=zf[:, :NF]
                    )
                    nc.sync.dma_start(
                        out=b2_d.rearrange("(p o) -> p o", o=8), in_=zf[:, :8]
                    )
                    nc.sync.dma_start(out=wc_d[:, :], in_=zf[:, :ntb])

            def body(it):
                b1t = cpool.tile([128, NF], F32, tag="b1t", name=f"b1t{it}")
                b2bc = cpool.tile([128, H], F32, tag="b2bc", name=f"b2bc{it}")
                wcs = cpool.tile([128, ntb], F32, tag="wcs", name=f"wcs{it}")

                def load_x(it, ck, xoff, tc_k, split=False):
                    xc = xp.tile([128, NH * tc_k], BF16, tag="xc", name=f"xc{it}_{ck}")
                    if split:
                        half = NH * tc_k // 2
                        nc.sync.dma_start(
                            out=xc[:, :half], in_=xr_d[:, xoff : xoff + half]
                        )
                        nc.sync.dma_start(
                            out=xc[:, half:],
                            in_=xr_d[:, xoff + half : xoff + NH * tc_k],
                        )
                    else:
                        nc.sync.dma_start(
                            out=xc, in_=xr_d[:, xoff : xoff + NH * tc_k]
                        )
                    return xc

                # chunk-0 x first so the PE can start immediately
                xc_next = load_x(it, 0, 0, chunks[0][1] * 128)
                nc.gpsimd.dma_start(
                    out=b1t, in_=b1_d.rearrange("(t p) -> p t", p=128)
                )

                # b2 broadcast across partitions via K=1 ones-vector matmul
                # (a partition_broadcast DMA costs ~114us of sequencer time)
                b2row = cpool.tile([1, H], F32, tag="b2row", name=f"b2row{it}")
                nc.sync.dma_start(
                    out=b2row, in_=b2_d.rearrange("(o n) -> o n", o=1)
                )
                ones1 = cpool.tile([1, 128], F32, tag="ones1", name=f"ones1{it}")
                nc.vector.memset(ones1, 1.0)
                for hh in range(2):
                    psb = ps1p.tile([128, 512], F32, tag="ps1", name=f"psb{it}_{hh}")
                    nc.tensor.matmul(
                        psb, lhsT=ones1, rhs=b2row[:, hh * 512 : (hh + 1) * 512],
                        start=True, stop=True,
                    )
                    nc.scalar.activation(
                        b2bc[:, hh * 512 : (hh + 1) * 512], psb, AFT.Copy
                    )

                # ---- resident weights: piecewise DMAs in consumption order,
                # first piece small (and on the otherwise-idle scalar queue)
                # so the PE starts early
                W1P = [(0, 1), (1, 7), (8, 8), (16, 8), (24, 8)]
                W1E = [nc.scalar, nc.scalar, nc.gpsimd, nc.sync, nc.scalar]
                W2P = [(0, 8), (8, 8), (16, 8), (24, 8)]
                W2E = [nc.gpsimd, nc.sync, nc.scalar, nc.gpsimd]
                w1pc, w2pc = {}, {}
                for (f0, nf), eng in zip(W1P, W1E):
                    t1 = w1p.tile(
                        [128, nf * NH * 128], BF16, tag=f"w1_{f0}",
                        name=f"w1_{it}_{f0}",
                    )
                    eng.dma_start(
                        out=t1,
                        in_=w1r_d[:, f0 * NH * 128 : (f0 + nf) * NH * 128],
                    )
                    w1pc[f0] = (t1, f0)
                nc.gpsimd.dma_start(out=wcs, in_=wc_d[:, :])
                for (f0, nf), eng in zip(W2P, W2E):
                    t2 = w2p.tile(
                        [128, nf * H], BF16, tag=f"w2_{f0}", name=f"w2_{it}_{f0}"
                    )
                    eng.dma_start(out=t2, in_=w2r_d[:, f0 * H : (f0 + nf) * H])
                    w2pc[f0] = (t2, f0)


                def _piece(pieces, f):
                    base = max(k for k in pieces if k <= f)
                    return pieces[base]

                def w1sl(f, h):   # lhsT tile (h, f) of w1
                    t, f0 = _piece(w1pc, f)
                    base = (f - f0) * NH * 128 + h * 128
                    return t[:, base : base + 128]

                def w2sl(f, hh):  # rhs slab (f, h-half) of w2
                    t, f0 = _piece(w2pc, f)
                    base = (f - f0) * H + hh * 512
                    return t[:, base : base + 512]

                xoff = 0
                for ck, (b0, nb) in enumerate(chunks):
                    tc_k = nb * 128
                    xc = xc_next
                    if ck + 1 < len(chunks):
                        nb_n = chunks[ck + 1][1]
                        xc_next = load_x(it, ck + 1, xoff + NH * tc_k, nb_n * 128)

                    # ---- GEMM1: hid[f, c] = gelu(w1.T x + b1), F on partitions
                    hids = []
                    for f in range(NF):
                        hid = hidp.tile(
                            [128, tc_k], BF16, tag=f"hid{f}", name=f"hid_{it}_{ck}_{f}"
                        )
                        if stages == "g2":
                            nc.vector.memset(hid, 0.01)
                            hids.append(hid)
                            continue
                        ps = ps1p.tile(
                            [128, tc_k], F32, tag="ps1", name=f"ps1_{it}_{ck}_{f}"
                        )
                        for h in range(NH):
                            nc.tensor.matmul(
                                ps,
                                lhsT=w1sl(f, h),
                                rhs=xc[:, h * tc_k : (h + 1) * tc_k],
                                start=(h == 0),
                                stop=(h == NH - 1),
                            )
                        nc.scalar.activation(
                            hid, ps, AFT.Gelu, bias=b1t[:, f : f + 1]
                        )
                        hids.append(hid)

                    if stages == "g1":
                        ot = outp.tile(
                            [128, tc_k], F32, tag="ot", name=f"otg1_{it}_{ck}"
                        )
                        nc.vector.tensor_copy(ot, hids[0])
                        nc.scalar.dma_start(
                            out=y_d[b0 * 128 : (b0 + 1) * 128, 0:tc_k], in_=ot
                        )
                        xoff += NH * tc_k
                        continue

                    # ---- GEMM2: y[c, h] = (hid.T w2 + b2) * wc
                    # One f-sweep over all nb*2 PSUM groups; each hid lhsT is
                    # used by its two h-half matmuls back-to-back so the
                    # second skips the stationary-operand reload.
                    pss = [
                        ps2p.tile(
                            [128, 512], F32, tag="ps2",
                            name=f"ps2_{it}_{ck}_{g}_{hh}",
                        )
                        for g in range(nb) for hh in range(2)
                    ]
                    for f in range(NF):
                        for g in range(nb):
                            for hh in range(2):
                                nc.tensor.matmul(
                                    pss[g * 2 + hh],
                                    lhsT=hids[f][:, g * 128 : (g + 1) * 128],
                                    rhs=w2sl(f, hh),
                                    start=(f == 0),
                                    stop=(f == NF - 1),
                                )
                    for g in range(nb):
                        for hh in range(2):
                            ot = outp.tile(
                                [128, 512], F32, tag="ot",
                                name=f"ot_{it}_{ck}_{hh}_{g}",
                            )
                            nc.vector.tensor_add(
                                ot, pss[g * 2 + hh],
                                b2bc[:, hh * 512 : (hh + 1) * 512],
                            )
                            nc.scalar.mul(ot, ot, wcs[:, b0 + g : b0 + g + 1])
                            (nc.gpsimd if g % 2 else nc.sync).dma_start(
                                out=y_d[
                                    (b0 + g) * 128 : (b0 + g + 1) * 128,
                                    hh * 512 : (hh + 1) * 512,
                                ],
                                in_=ot,
                            )
                    xoff += NH * tc_k

                if dram_internal:
                    yd = outp.tile([128, 8], F32, tag="ydum", name=f"ydum{it}")
                    nc.vector.memset(yd, 0.25)
                    nc.sync.dma_start(out=ydum_d[:, :], in_=yd)

            if loop_r is None:
                body(0)
            else:
                with tc.For_i(
                    0, loop_r, 1,
                    hint_engines=(
                        mybir.EngineType.PE,
                        mybir.EngineType.Activation,
                    ),
                ) as _i:
                    body(0)
    nc.compile()
    return nc


def _get_nc(ntb=NTB_DEFAULT):
    key = ("nc", ntb)
    if key not in _CACHE:
        _CACHE[key] = _build_nc(ntb)
    return _CACHE[key]


# ---------------------------------------------------------------- host side
def _route(x2d, router_w, cap):
    """Float64 mirror of the reference router. Returns per-expert padded
    index lists [E, cap] and combine weights [E, cap]."""
    logits = x2d.astype(np.float64) @ router_w.astype(np.float64).T  # [T, E]
    m = logits.max(axis=1, keepdims=True)
    p = np.exp(logits - m)
    p /= p.sum(axis=1, keepdims=True)
    # top-2 (ties -> lower index, matching jax.lax.top_k)
    order = np.argsort(-p, axis=1, kind="stable")
    top2 = order[:, :TOP_K]  # [T, 2]
    counts = np.zeros(E, np.int64)
    sel = np.zeros((T, E), bool)
    np.put_along_axis(sel, top2, True, axis=1)
    counts = sel.sum(axis=0)
    cap = max(cap, ((int(counts.max()) + 127) // 128) * 128)
    idx = np.zeros((E, cap), np.int64)
    wts = np.zeros((E, cap), np.float32)
    for e in range(E):
        tok = np.nonzero(sel[:, e])[0]
        n = len(tok)
        idx[e, :n] = tok
        wts[e, :n] = p[tok, e].astype(np.float32)
    return idx, wts, counts


def _prep_core_inputs(x2d, idx_e, wts_e, w1_e, b1_e, w2_e, b2_e):
    import ml_dtypes

    BF = ml_dtypes.bfloat16
    cap = idx_e.shape[0]
    ntb = cap // 128
    xg = x2d[idx_e].astype(BF)  # [C, H]
    # xr: chunk-major; within chunk ck of tc_k tokens:
    #   xr[p, off + h*tc_k + c] = xg[t0 + c, h*128 + p]
    slabs = []
    for b0, nb in _chunks(ntb):
        t0, tck = b0 * 128, nb * 128
        slabs.append(
            xg[t0 : t0 + tck]
            .reshape(tck, NH, 128)
            .transpose(2, 1, 0)
            .reshape(128, NH * tck)
        )
    xr = np.concatenate(slabs, axis=1)
    # w1r[p, f*NH*128 + h*128 + m] = w1[h*128 + p, f*128 + m]
    w1r = (
        w1_e.astype(BF)
        .reshape(NH, 128, NF, 128)
        .transpose(1, 2, 0, 3)
        .reshape(128, NF * NH * 128)
    )
    # w2r[p, f*H + n] = w2[f*128 + p, n]
    w2r = (
        w2_e.astype(BF)
        .reshape(NF, 128, H)
        .transpose(1, 0, 2)
        .reshape(128, NF * H)
    )
    wc = np.ascontiguousarray(wts_e.reshape(ntb, 128).T)  # [128, ntb]
    return {
        "xr": np.ascontiguousarray(xr),
        "w1r": np.ascontiguousarray(w1r),
        "w2r": np.ascontiguousarray(w2r),
        "b1": np.ascontiguousarray(b1_e, dtype=np.float32),
        "b2": np.ascontiguousarray(b2_e, dtype=np.float32),
        "wc": wc,
    }


def kernel(hidden_states, router_w, w1, b1, w2, b2):
    from concourse.bass_utils import run_bass_kernel_spmd

    x2d = np.ascontiguousarray(
        np.asarray(hidden_states, dtype=np.float32).reshape(T, H)
    )
    router_w = np.asarray(router_w, dtype=np.float32)
    w1 = np.asarray(w1, dtype=np.float32)
    b1 = np.asarray(b1, dtype=np.float32)
    w2 = np.asarray(w2, dtype=np.float32)
    b2 = np.asarray(b2, dtype=np.float32)

    idx, wts, counts = _route(x2d, router_w, NTB_DEFAULT * 128)
    ntb = idx.shape[1] // 128

    nc = _get_nc(ntb)
    in_maps = [
        _prep_core_inputs(x2d, idx[e], wts[e], w1[e], b1[e], w2[e], b2[e])
        for e in range(E)
    ]
    res = run_bass_kernel_spmd(nc, in_maps, core_ids=list(range(E)))

    out = np.zeros((T, H), np.float32)
    for e in range(E):
        n = int(counts[e])
        y = res.results[e]["y"]
        out[idx[e, :n]] += y[:n]
    return out.reshape(B, S, H)
